# revision 1
# baseline (speedup 1.0000x reference)
# Trainium2 Bass kernel for nn_MultiCondLayer:
#   out[b,o,n] = (sum_k (cond[b] @ W[k].T)[o,n] + sum_k b[k,o]) * x_mask[b,0,n]
# Key algebraic reduction: sum_k Linear_k(x) == Linear(x) with W' = sum_k W[k],
# b' = sum_k b[k]  (4x FLOP reduction vs. the naive einsum over k).
#
# Sharding: data-parallel over batch B=8 across the 8 NeuronCores (one batch
# element per core); the reduced [1024,1024] weight is replicated.
# Per-core compute: [1024c,4096n] activations x [1024c,1024o] weights as
# 512 PE matmuls (128x128 lhsT, 128x512 rhs, fp32r) accumulating in PSUM,
# evicted by a single fused DVE op: (psum + bias) * mask.

import numpy as np
from contextlib import ExitStack

import concourse.bass as bass
import concourse.mybir as mybir
import concourse.tile as tile
from concourse import bacc
from concourse.bass_utils import run_bass_kernel_spmd

P = 128
B, C, N = 8, 1024, 4096
O = 1024
NT = 512                 # matmul free dim = one fp32 PSUM bank
CO, OO, NN = C // P, O // P, N // NT
F32 = mybir.dt.float32
F32R = mybir.dt.float32r

N_CORES = 8


def build_module():
    nc = bacc.Bacc("TRN2", target_bir_lowering=False, debug=False,
                   num_devices=N_CORES)
    x = nc.dram_tensor("x", [C, N], F32R, kind="ExternalInput")    # cond[b]
    wt = nc.dram_tensor("wt", [C, O], F32R, kind="ExternalInput")  # (sum_k W[k]).T
    bv = nc.dram_tensor("bv", [O], F32, kind="ExternalInput")      # sum_k b[k]
    mk = nc.dram_tensor("mk", [N], F32, kind="ExternalInput")      # x_mask[b,0]
    out = nc.dram_tensor("out", [O, N], F32, kind="ExternalOutput")

    x_r = x.ap().rearrange("(c p) n -> p c n", p=P)      # [128, CO, N]
    wt_r = wt.ap().rearrange("(c p) o -> p c o", p=P)    # [128, CO, O]
    bv_r = bv.ap().rearrange("(j p) -> p j", p=P)        # [128, OO]
    mk_b = mk.ap()[None, :].broadcast_to([P, N])         # partition-bcast src

    with tile.TileContext(nc) as tc:
        with (
            tc.tile_pool(name="consts", bufs=1) as consts,
            tc.tile_pool(name="xs", bufs=2) as xs,
            tc.tile_pool(name="outs", bufs=4) as outs,
            tc.tile_pool(name="ps", bufs=8, space="PSUM") as psp,
        ):
            w_sb = consts.tile([P, CO, O], F32R)
            nc.sync.dma_start(w_sb[:], wt_r)
            bias_sb = consts.tile([P, OO], F32)
            nc.sync.dma_start(bias_sb[:], bv_r)
            mask_sb = consts.tile([P, N], F32)
            nc.sync.dma_start(mask_sb[:], mk_b)

            for n in range(NN):
                x_sb = xs.tile([P, CO, NT], F32R)
                nc.sync.dma_start(x_sb[:], x_r[:, :, n * NT:(n + 1) * NT])
                for o in range(OO):
                    ps = psp.tile([P, NT], F32)
                    for c in range(CO):
                        nc.tensor.matmul(
                            ps[:],
                            w_sb[:, c, o * P:(o + 1) * P],
                            x_sb[:, c, :],
                            start=(c == 0),
                            stop=(c == CO - 1),
                        )
                    ot = outs.tile([P, NT], F32)
                    nc.vector.scalar_tensor_tensor(
                        ot[:], ps[:], bias_sb[:, o:o + 1],
                        mask_sb[:, n * NT:(n + 1) * NT],
                        op0=mybir.AluOpType.add, op1=mybir.AluOpType.mult,
                    )
                    nc.sync.dma_start(
                        out.ap()[o * P:(o + 1) * P, n * NT:(n + 1) * NT], ot[:])
    nc.compile()
    return nc


_NC_CACHE = None


def _get_module():
    global _NC_CACHE
    if _NC_CACHE is None:
        _NC_CACHE = build_module()
    return _NC_CACHE


def _make_in_maps(cond, x_mask, W, b):
    wt = np.ascontiguousarray(W.sum(axis=0).T, dtype=np.float32)   # [C, O]
    bv = np.ascontiguousarray(b.sum(axis=0), dtype=np.float32)     # [O]
    in_maps = []
    for core in range(N_CORES):
        in_maps.append({
            "x": np.ascontiguousarray(cond[core], dtype=np.float32),
            "wt": wt,
            "bv": bv,
            "mk": np.ascontiguousarray(x_mask[core, 0], dtype=np.float32),
        })
    return in_maps


def run(cond, x_mask, W, b, trace=False, trace_cores=None):
    """Run on hardware; returns (out [B,O,N] fp32, BassKernelResults)."""
    nc = _get_module()
    in_maps = _make_in_maps(cond, x_mask, W, b)
    res = run_bass_kernel_spmd(
        nc, in_maps, core_ids=list(range(N_CORES)),
        trace=trace, trace_cores=trace_cores,
    )
    out = np.stack([res.results[i]["out"] for i in range(N_CORES)], axis=0)
    return out, res


def kernel(cond, x_mask, W, b):
    out, _ = run(cond, x_mask, W, b)
    return out
